# revision 1
# baseline (speedup 1.0000x reference)
"""Trainium2 Bass kernel for nn_DMRI2INetworkLayer (additive-attention pooling).

Reference (per batch row b):
    pre[s,h]  = X_item[b]@Wc + X_series[b,s]@We + pos[s]@Wp
    scores[s] = sum_h z[h]*tanh(pre[s,h])
    score_sum = sum_s where(mask, scores, 0)
    attn      = softmax(where(mask, scores, -inf))
    out[b]    = concat(sum_s attn[s]*X_series[b,s], score_sum)

Sharding: data-parallel over batch B=4096 across 8 NeuronCores (512 rows
per core). Host does layout/dtype marshalling only; all math on device.

Per-core device design (heavy operands bf16, f32 PSUM accumulation):
  - xt [128d, bc*200s] (b-major/s-minor cols) streams as rhs of the main
    matmul; lhsT=[We|We] in two PE col-groups -> pre-PSUM [(h x 2str), 400].
  - pos/item bias terms fold in as two more full-K accumulating matmuls:
    lhsT = [I64;0] / [0;I64] (zero-padded identities), rhs = pT2 (static)
    and cT via a step-0 broadcast AP. All matmuls keep K=128 (partial-row
    LDWEIGHTS compositions hang the HW).
  - tanh on ACT (PSUM -> SBUF bf16); z-dot via lhsT=[z;0|0;z] (M=32,
    zero-padded) matmuls col-packed 4x into one scores-PSUM tile.
  - scores: DVE drain -> 8 [2,200] reshape DMAs -> dense [bt,200] tiles
    (rows carry a fixed 16-row permutation); masked softmax on DVE/ACT.
  - weighted sum over s: per-(b, s-chunk) fused bf16 matmuls with the
    X-natural slice (K=128, s>=200 zero-padded) stationary and the attn^T
    column as rhs, accumulating into one PSUM bank [128d, bc] via
    per-element has_written semantics.
  - output: PE-transpose [d,b]->[b,d]; score_sum un-permuted on host.
"""
import os
import sys

sys.path.insert(0, "/opt/trn_rl_repo")

import numpy as np
import ml_dtypes
from contextlib import ExitStack

import concourse.bass as bass
import concourse.bacc as bacc
import concourse.tile as tile
from concourse import mybir
from concourse.bass_utils import run_bass_kernel_spmd

BF = mybir.dt.bfloat16
F32 = mybir.dt.float32
BF_NP = ml_dtypes.bfloat16

N_CORES = 8
B, S, D, H = 4096, 200, 128, 64
BC = B // N_CORES          # batch rows per core
SP = 256                   # padded S (two 128-row chunks; s>=200 zeroed)
GB = 4                     # b's per matmul group (2 col-group pairs)
TB = 16                    # b's per scores-PSUM tile (4 zz col-groups)

STAGE = os.environ.get("K_STAGE", "full")  # scores | softmax | attnT | full
REPEAT = int(os.environ.get("K_REPEAT", "1"))  # timing: repeat whole pipeline

# dense-scores row i within a 16-b block holds b16 = PERM16[i]
PERM16 = np.array([4 * (i // 4) + 2 * (i % 2) + ((i % 4) // 2) for i in range(16)])
PERM16_INV = np.argsort(PERM16)


def perm_full(bc):
    j = np.arange(bc)
    return (j // 16) * 16 + PERM16[j % 16]


_CACHE = {}


def build_nc(bc=BC):
    key = (bc, STAGE, REPEAT)
    if key in _CACHE:
        return _CACHE[key]
    bt_sz = min(128, bc)
    n_bt = bc // bt_sz
    n_tbt = bt_sz // TB        # 16-b tiles per softmax tile

    nc = bacc.Bacc("TRN2", target_bir_lowering=False, num_devices=N_CORES)

    xt = nc.declare_dram_parameter("xt", [D, bc * S], BF, isOutput=False)
    xn0 = nc.declare_dram_parameter("xn0", [128, bc * D], BF, isOutput=False)
    xn1 = nc.declare_dram_parameter("xn1", [128, bc * D], BF, isOutput=False)
    wew = nc.declare_dram_parameter("wew", [D, 128], BF, isOutput=False)
    ii = nc.declare_dram_parameter("ii", [128, 128], BF, isOutput=False)
    zz = nc.declare_dram_parameter("zz", [128, 32], BF, isOutput=False)
    ptab = nc.declare_dram_parameter("ptab", [D, S], BF, isOutput=False)
    wpw = nc.declare_dram_parameter("wpw", [D, 128], BF, isOutput=False)
    xitT = nc.declare_dram_parameter("xitT", [D, bc], BF, isOutput=False)
    wcw = nc.declare_dram_parameter("wcw", [D, 128], BF, isOutput=False)
    idbf = nc.declare_dram_parameter("idbf", [128, 128], BF, isOutput=False)
    idf = nc.declare_dram_parameter("idf", [128, 128], F32, isOutput=False)
    mbp = nc.declare_dram_parameter("mbp", [bc, S], F32, isOutput=False)
    m01p = nc.declare_dram_parameter("m01p", [bc, S], F32, isOutput=False)
    out_attn = nc.declare_dram_parameter("out_attn", [bc, D], F32, isOutput=True)
    out_ssum = nc.declare_dram_parameter("out_ssum", [bc, 1], F32, isOutput=True)

    with tile.TileContext(nc) as tc, ExitStack() as ctx:
        const = ctx.enter_context(tc.tile_pool(name="const", bufs=1))
        xtp = ctx.enter_context(tc.tile_pool(name="xtp", bufs=3))
        xnp = ctx.enter_context(tc.tile_pool(name="xnp", bufs=n_tbt + 2))
        thp = ctx.enter_context(tc.tile_pool(name="thp", bufs=6))
        scp = ctx.enter_context(tc.tile_pool(name="scp", bufs=3))
        smp = ctx.enter_context(tc.tile_pool(name="smp", bufs=2))
        atp = ctx.enter_context(tc.tile_pool(name="atp", bufs=2))
        outp = ctx.enter_context(tc.tile_pool(name="outp", bufs=2))
        pre_ps = ctx.enter_context(tc.tile_pool(name="pre_ps", bufs=2, space="PSUM"))
        sc_ps = ctx.enter_context(tc.tile_pool(name="sc_ps", bufs=2, space="PSUM"))
        o5_ps = ctx.enter_context(tc.tile_pool(name="o5_ps", bufs=1, space="PSUM"))
        t_ps = ctx.enter_context(tc.tile_pool(name="t_ps", bufs=1, space="PSUM"))

        # ---------- constants ----------
        def cdma(shape, dt_, src, tag):
            t = const.tile(shape, dt_, tag=tag)
            nc.sync.dma_start(t[:], src)
            return t

        wew_t = cdma([D, 128], BF, wew[:], "wew_t")
        ii_t = cdma([128, 128], BF, ii[:], "ii_t")   # [:,0:64]=[I64;0], [:,64:]=[0;I64]
        zz_t = cdma([128, 32], BF, zz[:], "zz_t")
        pos_t = cdma([D, S], BF, ptab[:], "pos_t")
        wpw_t = cdma([D, 128], BF, wpw[:], "wpw_t")
        xitT_t = cdma([D, bc], BF, xitT[:], "xitT_t")
        wcw_t = cdma([D, 128], BF, wcw[:], "wcw_t")
        idbf_t = cdma([128, 128], BF, idbf[:], "idbf_t")
        idf_t = cdma([128, 128], F32, idf[:], "idf_t")

        # ---------- phase 0: pT2 (rows 0-63, rows 64-127 zero) ----------
        ph_ps = t_ps.tile([128, 512], F32, tag="tps")
        nc.tensor.matmul(ph_ps[0:64, 0:S], wpw_t[:, 0:64], pos_t[:],
                         start=True, stop=True, tile_position=(0, 0),
                         skip_group_check=True)
        pt2 = const.tile([128, 2 * S], BF, tag="pt2")
        nc.vector.memset(pt2[:], 0.0)
        nc.vector.tensor_copy(pt2[0:64, 0:S], ph_ps[0:64, 0:S])
        nc.vector.tensor_copy(pt2[0:64, S:2 * S], ph_ps[0:64, 0:S])

        # cT at rows 64-127 (rows 0-63 zero)
        ct = const.tile([128, bc], BF, tag="ct")
        nc.vector.memset(ct[:], 0.0)
        for j in range((bc + 511) // 512):
            n = min(512, bc - j * 512)
            c_ps = t_ps.tile([128, 512], F32, tag="tps")
            nc.tensor.matmul(c_ps[64:128, 0:n], wcw_t[:, 64:128],
                             xitT_t[:, j * 512:j * 512 + n],
                             start=True, stop=True, tile_position=(0, 64),
                             skip_group_check=True)
            nc.vector.tensor_copy(ct[64:128, j * 512:j * 512 + n], c_ps[64:128, 0:n])

        # ---------- step5 accumulator ----------
        o5 = o5_ps.tile([D, bc], F32)
        nc.vector.memset(o5[:], 0.0)
        step5_n = 0

        for bt_rep in range(n_bt * REPEAT):
            bt = bt_rep % n_bt
            sc_dense = smp.tile([bt_sz, S], F32, tag="sc_dense")
            xn_tiles = []
            for tb_i in range(n_tbt):
                tb = bt * n_tbt + tb_i
                xt_t = xtp.tile([D, TB * S], BF, tag="xt_t")
                nc.sync.dma_start(xt_t[:], xt[:, tb * TB * S:(tb + 1) * TB * S])
                xn0_t = xnp.tile([128, TB * D], BF, tag="xn0_t")
                nc.sync.dma_start(xn0_t[:], xn0[:, tb * TB * D:(tb + 1) * TB * D])
                xn1_t = xnp.tile([128, TB * D], BF, tag="xn1_t")
                nc.sync.dma_start(xn1_t[:], xn1[:, tb * TB * D:(tb + 1) * TB * D])
                xn_tiles.append((xn0_t, xn1_t))

                sc_psum = sc_ps.tile([128, 2 * S], F32, tag="sc_psum")
                for g in range(TB // GB):
                    pre = pre_ps.tile([128, 2 * S], F32, tag="pre")
                    c0 = g * GB * S
                    b0 = tb * TB + g * GB
                    ctA = ct[:, b0:b0 + 2, None].broadcast_to((128, 2, S))
                    ctB = ct[:, b0 + 2:b0 + 4, None].broadcast_to((128, 2, S))
                    # rows 0-63 of pre: b0, b0+1
                    nc.tensor.matmul(pre[0:64, :], wew_t[:, 0:64],
                                     xt_t[:, c0:c0 + 2 * S],
                                     start=True, stop=False, tile_position=(0, 0),
                                     skip_group_check=True)
                    nc.tensor.matmul(pre[0:64, :], ii_t[:, 0:64], pt2[:],
                                     start=False, stop=False, tile_position=(0, 0),
                                     skip_group_check=True)
                    nc.tensor.matmul(pre[0:64, :], ii_t[:, 64:128], ctA,
                                     start=False, stop=True, tile_position=(0, 0),
                                     skip_group_check=True)
                    # rows 64-127 of pre: b0+2, b0+3
                    nc.tensor.matmul(pre[64:128, :], wew_t[:, 64:128],
                                     xt_t[:, c0 + 2 * S:c0 + 4 * S],
                                     start=True, stop=False, tile_position=(0, 64),
                                     skip_group_check=True)
                    nc.tensor.matmul(pre[64:128, :], ii_t[:, 0:64], pt2[:],
                                     start=False, stop=False, tile_position=(0, 64),
                                     skip_group_check=True)
                    nc.tensor.matmul(pre[64:128, :], ii_t[:, 64:128], ctB,
                                     start=False, stop=True, tile_position=(0, 64),
                                     skip_group_check=True)
                    th = thp.tile([128, 2 * S], BF, tag="th")
                    nc.scalar.activation(th[:], pre[:],
                                         mybir.ActivationFunctionType.Tanh)
                    nc.tensor.matmul(sc_psum[32 * g:32 * g + 32, :], zz_t[:], th[:],
                                     start=True, stop=True,
                                     tile_position=(0, 32 * g),
                                     skip_group_check=True)
                sc_sp = scp.tile([128, 2 * S], F32, tag="sc_sp")
                nc.vector.tensor_copy(sc_sp[:], sc_psum[:])
                # scatter row-pairs {32g,32g+1} x (b01,s) -> dense rows 4g+2h+r
                r0 = tb_i * TB
                for g in range(4):
                    for h in range(2):
                        src = sc_sp[32 * g:32 * g + 2, h * S:(h + 1) * S]
                        dr = r0 + 4 * g + 2 * h
                        nc.sync.dma_start(sc_dense[dr:dr + 2, :], src)

            # ---------- masked softmax ----------
            if STAGE == "scores":
                nc.sync.dma_start(out_attn[bt * bt_sz:(bt + 1) * bt_sz, :],
                                  sc_dense[:, 0:D])
                zs = smp.tile([bt_sz, 1], F32, tag="zs")
                nc.vector.memset(zs[:], 0.0)
                nc.sync.dma_start(out_ssum[bt * bt_sz:(bt + 1) * bt_sz, :], zs[:])
                continue
            mb_t = smp.tile([bt_sz, S], F32, tag="mb_t")
            nc.sync.dma_start(mb_t[:], mbp[bt * bt_sz:(bt + 1) * bt_sz, :])
            m01_t = smp.tile([bt_sz, S], F32, tag="m01_t")
            nc.sync.dma_start(m01_t[:], m01p[bt * bt_sz:(bt + 1) * bt_sz, :])

            sc_m = smp.tile([bt_sz, S], F32, tag="sc_m")
            ssum = smp.tile([bt_sz, 1], F32, tag="ssum")
            nc.vector.tensor_mul(sc_m[:], sc_dense[:], m01_t[:])
            nc.vector.reduce_sum(ssum[:], sc_m[:], axis=mybir.AxisListType.X)
            sc_soft = smp.tile([bt_sz, S], F32, tag="sc_soft")
            nc.vector.tensor_add(sc_soft[:], sc_m[:], mb_t[:])
            nmax = smp.tile([bt_sz, 1], F32, tag="nmax")
            nc.vector.tensor_reduce(nmax[:], sc_soft[:], axis=mybir.AxisListType.X,
                                    op=mybir.AluOpType.max, negate=True)
            expd = smp.tile([bt_sz, S], F32, tag="expd")
            nc.scalar.activation(expd[:], sc_soft[:],
                                 mybir.ActivationFunctionType.Exp,
                                 bias=nmax[:], scale=1.0)
            den = smp.tile([bt_sz, 1], F32, tag="den")
            nc.vector.reduce_sum(den[:], expd[:], axis=mybir.AxisListType.X)
            rden = smp.tile([bt_sz, 1], F32, tag="rden")
            nc.vector.reciprocal(rden[:], den[:])
            attn = atp.tile([bt_sz, SP], BF, tag="attn")
            nc.vector.memset(attn[:], 0.0)
            nc.vector.tensor_scalar_mul(attn[:, 0:S], expd[:], rden[:])
            nc.sync.dma_start(out_ssum[bt * bt_sz:(bt + 1) * bt_sz, :], ssum[:])

            if STAGE == "softmax":
                nc.sync.dma_start(out_attn[bt * bt_sz:(bt + 1) * bt_sz, :],
                                  expd[:, 0:D])
                continue

            # ---------- attn^T (two full 128-col blocks) ----------
            at_ps = t_ps.tile([128, 128], BF, tag="tps_bf")
            nc.tensor.transpose(at_ps[0:128, 0:bt_sz], attn[:, 0:128],
                                idbf_t[0:bt_sz, 0:bt_sz])
            atT_lo = atp.tile([128, bt_sz], BF, tag="atT_lo")
            nc.vector.tensor_copy(atT_lo[:], at_ps[0:128, 0:bt_sz])
            at_ps2 = t_ps.tile([128, 128], BF, tag="tps_bf")
            nc.tensor.transpose(at_ps2[0:128, 0:bt_sz], attn[:, 128:256],
                                idbf_t[0:bt_sz, 0:bt_sz])
            atT_hi = atp.tile([128, bt_sz], BF, tag="atT_hi")
            nc.vector.tensor_copy(atT_hi[:], at_ps2[0:128, 0:bt_sz])

            # ---------- weighted sum over s ----------
            if STAGE == "attnT":
                ats = outp.tile([128, bt_sz], F32, tag="ats")
                nc.vector.tensor_copy(ats[:], atT_lo[:])
                nc.sync.dma_start(out_attn[bt * bt_sz:(bt + 1) * bt_sz, :],
                                  ats[0:bt_sz, 0:D])
                continue
            for tb_i in range(n_tbt):
                xn0_t, xn1_t = xn_tiles[tb_i]
                for bi in range(TB):
                    b_in_bt = tb_i * TB + bi
                    b_loc = bt * bt_sz + b_in_bt
                    j = (b_in_bt // 16) * 16 + int(PERM16_INV[b_in_bt % 16])
                    step5_n += 2
                    nc.tensor.matmul(o5[:, b_loc:b_loc + 1],
                                     xn0_t[:, bi * D:(bi + 1) * D],
                                     atT_lo[:, j:j + 1],
                                     start=False, stop=False,
                                     skip_group_check=True)
                    nc.tensor.matmul(o5[:, b_loc:b_loc + 1],
                                     xn1_t[:, bi * D:(bi + 1) * D],
                                     atT_hi[:, j:j + 1],
                                     start=False, stop=(step5_n == 2 * bc * REPEAT),
                                     skip_group_check=True)

        # ---------- drain weighted sum, transpose to [b, d], store ----------
        if STAGE == "full":
            o5_s = outp.tile([D, bc], F32, tag="o5_s")
            nc.vector.tensor_copy(o5_s[:], o5[:])
            for t in range((bc + 127) // 128):
                n = min(128, bc - t * 128)
                ot_ps = t_ps.tile([128, 128], F32, tag="otps")
                nc.tensor.transpose(ot_ps[0:n, :], o5_s[:, t * 128:t * 128 + n],
                                    idf_t[:])
                ob = outp.tile([128, D], F32, tag="ob")
                nc.vector.tensor_copy(ob[0:n, :], ot_ps[0:n, :])
                nc.sync.dma_start(out_attn[t * 128:t * 128 + n, :], ob[0:n, :])
        else:
            o5_d = outp.tile([D, bc], F32, tag="o5_s")
            nc.vector.tensor_copy(o5_d[:], o5[:])

    nc.compile()
    _CACHE[key] = nc
    return nc


def _prep_core(Xs, Xit, pos, mask, We, Wp, Wc, z, bc):
    """Host-side marshalling (layout/dtype only) for one core's shard."""
    d = {}
    d["xt"] = np.ascontiguousarray(Xs.transpose(2, 0, 1).reshape(D, bc * S)).astype(BF_NP)
    xn = Xs.transpose(1, 0, 2)                     # [S, bc, D]
    d["xn0"] = np.ascontiguousarray(xn[0:128].reshape(128, bc * D)).astype(BF_NP)
    xn1 = np.zeros((128, bc, D), np.float32)
    xn1[0:S - 128] = xn[128:S]
    d["xn1"] = xn1.reshape(128, bc * D).astype(BF_NP)
    d["wew"] = np.concatenate([We, We], 1).astype(BF_NP)
    i64 = np.eye(64, dtype=np.float32)
    iim = np.zeros((128, 128), np.float32)
    iim[0:64, 0:64] = i64          # [I64; 0] for the pos fold
    iim[64:128, 64:128] = i64      # [0; I64] for the item fold
    d["ii"] = iim.astype(BF_NP)
    zzm = np.zeros((128, 32), np.float32)
    zzm[0:64, 0] = z
    zzm[64:128, 1] = z
    d["zz"] = zzm.astype(BF_NP)
    d["ptab"] = np.ascontiguousarray(pos.T).astype(BF_NP)
    d["wpw"] = np.concatenate([Wp, Wp], 1).astype(BF_NP)
    d["xitT"] = np.ascontiguousarray(Xit.T).astype(BF_NP)
    d["wcw"] = np.concatenate([Wc, Wc], 1).astype(BF_NP)
    d["idbf"] = np.eye(128, dtype=np.float32).astype(BF_NP)
    d["idf"] = np.eye(128, dtype=np.float32)
    pf = perm_full(bc)
    m01 = mask.astype(np.float32)
    d["m01p"] = np.ascontiguousarray(m01[pf])
    d["mbp"] = np.ascontiguousarray((m01[pf] - 1.0) * 1.0e30)
    return d


def _unshard(results, bc):
    pf = perm_full(bc)
    outs = []
    for k in range(len(results)):
        attn_out = results[k]["out_attn"]
        ssum_perm = results[k]["out_ssum"]
        ssum = np.empty_like(ssum_perm)
        ssum[pf] = ssum_perm
        outs.append(np.concatenate([attn_out, ssum], axis=1))
    return np.concatenate(outs, axis=0)


def make_in_maps(X_series, pos_series, X_item, valid_mask, Wc, Wp, We, z, bc):
    in_maps = []
    for k in range(N_CORES):
        sl = slice(k * bc, (k + 1) * bc)
        in_maps.append(_prep_core(np.asarray(X_series[sl], np.float32),
                                  np.asarray(X_item[sl], np.float32),
                                  np.asarray(pos_series, np.float32),
                                  np.asarray(valid_mask[sl]),
                                  np.asarray(We, np.float32),
                                  np.asarray(Wp, np.float32),
                                  np.asarray(Wc, np.float32),
                                  np.asarray(z, np.float32), bc))
    return in_maps


def kernel(X_series, pos_series, X_item, valid_mask, Wc, Wp, We, z):
    X_series = np.asarray(X_series, np.float32)
    bc = X_series.shape[0] // N_CORES
    nc = build_nc(bc)
    in_maps = make_in_maps(X_series, pos_series, X_item, valid_mask,
                           Wc, Wp, We, z, bc)
    res = run_bass_kernel_spmd(nc, in_maps, list(range(N_CORES)))
    return _unshard(res.results, bc)



# revision 10
# speedup vs baseline: 1.6205x; 1.6205x over previous
"""Trainium2 Bass kernel for nn_DMRI2INetworkLayer (additive-attention pooling).

Reference (per batch row b):
    pre[s,h]  = X_item[b]@Wc + X_series[b,s]@We + pos[s]@Wp
    scores[s] = sum_h z[h]*tanh(pre[s,h])
    score_sum = sum_s where(mask, scores, 0)
    attn      = softmax(where(mask, scores, -inf))
    out[b]    = concat(sum_s attn[s]*X_series[b,s], score_sum)

Sharding: data-parallel over batch B=4096 across 8 NeuronCores (512 rows
per core). Host does layout/dtype marshalling only; all math on device.

Per-core design (s-major scores phase, [s,b] softmax, fp8 weighted sum):
  - xsm [128d, (s,b)] bf16 streams as moving operand; per s-pair tile the
    two We matmuls (N=512, PE col-groups (0,0)/(0,64)) accumulate onto a
    PSUM bank pre-initialized with the item bias c2 = Wc^T X_item^T (DVE
    copy), so no identity-fold matmuls are needed.
  - tanh on ACT with the pos bias pb[:,t] = [Wp^T pos_even; Wp^T pos_odd]
    as the per-partition activation bias (fused, zero extra cost).
  - z-dot via a block-diagonal stationary zzbig [128, 16*32]: 16 s-pair
    tiles accumulate into one 32-row PSUM group, landing scores directly
    in [s, b] layout (no scatter DMAs, no host permutation).
  - softmax in [s, b]: exp without max-subtraction (|scores| <~ 6 is f32
    safe; bias=-ln16 keeps fp8 attn in range), mask multiply, and
    partition-direction sums (den, score_sum) via ones-stationary matmuls.
  - weighted sum over s: per-b fp8 matmuls X_b^T[s,d] @ attn_b[s,1]
    accumulating into one PSUM bank [128d, 512b]; 1/den folded into the
    final per-partition scale after the PE transpose to [b, d].
  - X ships once per layout: xsm bf16 (26MB) + xn fp8 (13MB) per core.
"""
import os
import sys

sys.path.insert(0, "/opt/trn_rl_repo")

import numpy as np
import ml_dtypes
from contextlib import ExitStack

import concourse.bass as bass
import concourse.bacc as bacc
import concourse.tile as tile
from concourse import mybir
from concourse.bass_utils import run_bass_kernel_spmd

BF = mybir.dt.bfloat16
F32 = mybir.dt.float32
FP8 = mybir.dt.float8e4 if os.environ.get('K_FP8','0')=='1' else mybir.dt.bfloat16
BF_NP = ml_dtypes.bfloat16
FP8_NP = ml_dtypes.float8_e4m3 if os.environ.get('K_FP8','0')=='1' else ml_dtypes.bfloat16

N_CORES = 8
B, S, D, H = 4096, 200, 128, 64
BC = B // N_CORES          # batch rows per core
NT = S // 2                # s-pair tiles (100)
S0 = 128                   # s rows in bank0
S1 = S - S0                # s rows in bank1 (72)
LN16 = float(np.log(16.0))

_CACHE = {}


def build_nc(bc=BC):
    if bc in _CACHE:
        return _CACHE[bc]
    assert bc == 512, "layout hardcoded for bc=512"
    nb = bc // 128             # 128-b output chunks (4)

    nc = bacc.Bacc("TRN2", target_bir_lowering=False, num_devices=N_CORES)

    xsm = nc.declare_dram_parameter("xsm", [D, S * bc], BF, isOutput=False)
    xn0 = nc.declare_dram_parameter("xn0", [128, bc * D], FP8, isOutput=False)
    xn1 = nc.declare_dram_parameter("xn1", [128, bc * D], FP8, isOutput=False)
    m01a = nc.declare_dram_parameter("m01a", [128, bc], F32, isOutput=False)
    m01b = nc.declare_dram_parameter("m01b", [128, bc], F32, isOutput=False)
    we2 = nc.declare_dram_parameter("we2", [D, 128], BF, isOutput=False)
    wc2 = nc.declare_dram_parameter("wc2", [D, 128], BF, isOutput=False)
    wp2 = nc.declare_dram_parameter("wp2", [D, 128], BF, isOutput=False)
    pos_ev = nc.declare_dram_parameter("pos_ev", [D, NT], BF, isOutput=False)
    pos_od = nc.declare_dram_parameter("pos_od", [D, NT], BF, isOutput=False)
    zzbig = nc.declare_dram_parameter("zzbig", [128, 512], BF, isOutput=False)
    ones32 = nc.declare_dram_parameter("ones32", [128, 1], F32, isOutput=False)
    ones8 = nc.declare_dram_parameter("ones8", [128, 1], FP8, isOutput=False)
    idf = nc.declare_dram_parameter("idf", [128, 128], F32, isOutput=False)
    xitT = nc.declare_dram_parameter("xitT", [D, bc], BF, isOutput=False)
    out_attn = nc.declare_dram_parameter("out_attn", [bc, D], F32, isOutput=True)
    out_ssum = nc.declare_dram_parameter("out_ssum", [bc, 1], F32, isOutput=True)

    with tile.TileContext(nc) as tc, ExitStack() as ctx:
        const = ctx.enter_context(tc.tile_pool(name="const", bufs=1))
        xsp = ctx.enter_context(tc.tile_pool(name="xsp", bufs=3))
        thp = ctx.enter_context(tc.tile_pool(name="thp", bufs=4))
        smp = ctx.enter_context(tc.tile_pool(name="smp", bufs=1))
        outp = ctx.enter_context(tc.tile_pool(name="outp", bufs=2))
        xnp = ctx.enter_context(tc.tile_pool(name="xnp", bufs=3))
        pre_ps = ctx.enter_context(tc.tile_pool(name="pre_ps", bufs=2, space="PSUM"))
        sc_ps = ctx.enter_context(tc.tile_pool(name="sc_ps", bufs=1, space="PSUM"))
        o5_ps = ctx.enter_context(tc.tile_pool(name="o5_ps", bufs=1, space="PSUM"))
        t_ps = ctx.enter_context(tc.tile_pool(name="t_ps", bufs=2, space="PSUM"))

        # ---------- constants (scalar DMA queue; sync queue streams xsm) ----
        def cdma(shape, dt_, src, tag):
            t = const.tile(shape, dt_, tag=tag)
            nc.scalar.dma_start(t[:], src)
            return t

        we2_t = cdma([D, 128], BF, we2[:], "we2_t")
        wc2_t = cdma([D, 128], BF, wc2[:], "wc2_t")
        wp2_t = cdma([D, 128], BF, wp2[:], "wp2_t")
        pev_t = cdma([D, NT], BF, pos_ev[:], "pev_t")
        pod_t = cdma([D, NT], BF, pos_od[:], "pod_t")
        zz_t = cdma([128, 512], BF, zzbig[:], "zz_t")
        on32_t = cdma([128, 1], F32, ones32[:], "on32_t")
        on8_t = cdma([128, 1], FP8, ones8[:], "on8_t")
        idf_t = cdma([128, 128], F32, idf[:], "idf_t")
        xitT_t = cdma([D, bc], BF, xitT[:], "xitT_t")
        m01a_t = cdma([128, bc], F32, m01a[:], "m01a_t")
        m01b_t = cdma([128, bc], F32, m01b[:], "m01b_t")

        # ---------- on-chip small precomputes ----------
        # c2[128, bc]: rows 0-63 and 64-127 both = (Wc^T X_item^T)[h, b]
        c2_ps = t_ps.tile([128, bc], F32, tag="tps")
        nc.tensor.matmul(c2_ps[0:64, :], wc2_t[:, 0:64], xitT_t[:],
                         start=True, stop=True, tile_position=(0, 0),
                         skip_group_check=True)
        nc.tensor.matmul(c2_ps[64:128, :], wc2_t[:, 64:128], xitT_t[:],
                         start=True, stop=True, tile_position=(0, 64),
                         skip_group_check=True)
        c2_sb = const.tile([128, bc], F32, tag="c2_sb")
        nc.vector.tensor_copy(c2_sb[:], c2_ps[:])

        # pb[128, NT]: col t = [Wp^T pos[2t]; Wp^T pos[2t+1]]
        pb_ps = t_ps.tile([128, NT], F32, tag="tps")
        nc.tensor.matmul(pb_ps[0:64, :], wp2_t[:, 0:64], pev_t[:],
                         start=True, stop=True, tile_position=(0, 0),
                         skip_group_check=True)
        nc.tensor.matmul(pb_ps[64:128, :], wp2_t[:, 64:128], pod_t[:],
                         start=True, stop=True, tile_position=(0, 64),
                         skip_group_check=True)
        pb_sb = const.tile([128, NT], F32, tag="pb_sb")
        nc.vector.tensor_copy(pb_sb[:], pb_ps[:])

        # persistent PSUM: scores banks, weighted-sum accumulator
        sc0 = sc_ps.tile([128, bc], F32, tag="sc0")
        sc1 = sc_ps.tile([128, bc], F32, tag="sc1")
        nc.vector.memset(sc1[96:128, :], 0.0)   # rows never written by MMs
        o5 = o5_ps.tile([D, bc], F32, tag="o5")
        nc.vector.memset(o5[:], 0.0)
        dsum_sb = const.tile([128, bc], F32, tag="dsum_sb")
        nc.vector.memset(dsum_sb[:], 0.0)
        expb = const.tile([128, 1], F32, tag="expb")
        nc.vector.memset(expb[:], -LN16)

        # ---------- phase 1: scores in [s, b] ----------
        CH = 4                       # s-pair tiles per DMA chunk
        for chunk in range(NT // CH):
            xt = xsp.tile([128, CH * 2 * bc], BF, tag="xt")
            nc.sync.dma_start(xt[:], xsm[:, chunk * CH * 2 * bc:
                                         (chunk + 1) * CH * 2 * bc])
            for j in range(CH):
                t = chunk * CH + j
                xe = xt[:, (2 * j) * bc:(2 * j + 1) * bc]
                xo = xt[:, (2 * j + 1) * bc:(2 * j + 2) * bc]
                pre = pre_ps.tile([128, bc], F32, tag="pre")
                nc.vector.tensor_copy(pre[:], c2_sb[:])
                nc.tensor.matmul(pre[0:64, :], we2_t[:, 0:64], xe,
                                 start=False, stop=False, tile_position=(0, 0),
                                 skip_group_check=True)
                nc.tensor.matmul(pre[64:128, :], we2_t[:, 64:128], xo,
                                 start=False, stop=True, tile_position=(0, 64),
                                 skip_group_check=True)
                th = thp.tile([128, bc], BF, tag="th")
                nc.scalar.activation(th[:], pre[:],
                                     mybir.ActivationFunctionType.Tanh,
                                     bias=pb_sb[:, t:t + 1])
                g, jj = divmod(t, 16)
                tgt, ro = (sc0, 32 * g) if g < 4 else (sc1, 32 * (g - 4))
                nc.tensor.matmul(tgt[ro:ro + 32, :],
                                 zz_t[:, 32 * jj:32 * jj + 32], th[:],
                                 start=(jj == 0), stop=(jj == 15 or t == NT - 1),
                                 tile_position=(0, ro), skip_group_check=True)

        # ---------- phase 2: masked softmax pieces in [s, b] ----------
        ms0 = smp.tile([128, bc], F32, tag="ms0")
        nc.vector.tensor_mul(ms0[:], sc0[:], m01a_t[:])
        ms1 = smp.tile([128, bc], F32, tag="ms1")
        nc.vector.tensor_mul(ms1[:], sc1[:], m01b_t[:])
        e0 = smp.tile([128, bc], F32, tag="e0")
        nc.scalar.activation(e0[:], sc0[:], mybir.ActivationFunctionType.Exp,
                             bias=expb[:])
        e1 = smp.tile([128, bc], F32, tag="e1")
        nc.scalar.activation(e1[:], sc1[:], mybir.ActivationFunctionType.Exp,
                             bias=expb[:])
        att0 = smp.tile([128, bc], FP8, tag="att0")
        nc.vector.tensor_mul(att0[:], e0[:], m01a_t[:])
        att1 = smp.tile([128, bc], FP8, tag="att1")
        nc.vector.tensor_mul(att1[:], e1[:], m01b_t[:])

        dsum = t_ps.tile([128, bc], F32, tag="tps")
        nc.tensor.matmul(dsum[0:1, :], on8_t[:], att0[:],
                         start=True, stop=False, tile_position=(0, 0),
                         skip_group_check=True)
        nc.tensor.matmul(dsum[0:1, :], on8_t[:], att1[:],
                         start=False, stop=True, tile_position=(0, 0),
                         skip_group_check=True)
        nc.tensor.matmul(dsum[32:33, :], on32_t[:], ms0[:],
                         start=True, stop=False, tile_position=(0, 32),
                         skip_group_check=True)
        nc.tensor.matmul(dsum[32:33, :], on32_t[:], ms1[:],
                         start=False, stop=True, tile_position=(0, 32),
                         skip_group_check=True)
        nc.vector.tensor_copy(dsum_sb[0:1, :], dsum[0:1, :])
        nc.vector.tensor_copy(dsum_sb[32:33, :], dsum[32:33, :])

        rdens = []
        for c in range(nb):
            dt_ps = t_ps.tile([128, 128], F32, tag="tps")
            nc.tensor.transpose(dt_ps[:], dsum_sb[:, c * 128:(c + 1) * 128],
                                idf_t[:])
            rden = smp.tile([128, 1], F32, tag="rden", bufs=4)
            nc.vector.reciprocal(rden[:], dt_ps[:, 0:1])
            rdens.append(rden)
            ssc = smp.tile([128, 1], F32, tag="ssc", bufs=2)
            nc.vector.tensor_copy(ssc[:], dt_ps[:, 32:33])
            nc.sync.dma_start(out_ssum[c * 128:(c + 1) * 128, :], ssc[:])

        # ---------- phase 3: weighted sum over s ----------
        n5 = 0
        XB = 64
        for ch in range(bc // XB):
            xn0_c = xnp.tile([128, XB * D], FP8, tag="xn0_c")
            nc.scalar.dma_start(xn0_c[:], xn0[:, ch * XB * D:(ch + 1) * XB * D])
            xn1_c = xnp.tile([128, XB * D], FP8, tag="xn1_c")
            nc.scalar.dma_start(xn1_c[:], xn1[:, ch * XB * D:(ch + 1) * XB * D])
            for i in range(XB):
                b = ch * XB + i
                n5 += 2
                nc.tensor.matmul(o5[:, b:b + 1], xn0_c[:, i * D:(i + 1) * D],
                                 att0[:, b:b + 1], start=False, stop=False,
                                 skip_group_check=True)
                nc.tensor.matmul(o5[:, b:b + 1], xn1_c[:, i * D:(i + 1) * D],
                                 att1[:, b:b + 1], start=False, stop=(n5 == 2 * bc),
                                 skip_group_check=True)

        # ---------- output: transpose [d, b] -> [b, d], scale by 1/den ----
        o5_s = outp.tile([D, bc], F32, tag="o5_s")
        nc.vector.tensor_copy(o5_s[:], o5[:])
        for c in range(nb):
            ot_ps = t_ps.tile([128, 128], F32, tag="tps")
            nc.tensor.transpose(ot_ps[:], o5_s[:, c * 128:(c + 1) * 128],
                                idf_t[:])
            ob = outp.tile([128, D], F32, tag="ob")
            nc.vector.tensor_scalar_mul(ob[:], ot_ps[:], rdens[c][:])
            nc.sync.dma_start(out_attn[c * 128:(c + 1) * 128, :], ob[:])

    nc.compile()
    _CACHE[bc] = nc
    return nc


def _prep_core(Xs, Xit, pos, mask, We, Wp, Wc, z, bc):
    """Host-side marshalling (layout/dtype only) for one core's shard."""
    d = {}
    d["xsm"] = np.ascontiguousarray(
        Xs.transpose(2, 1, 0).reshape(D, S * bc)).astype(BF_NP)
    xn = Xs.transpose(1, 0, 2)                     # [S, bc, D]
    d["xn0"] = np.ascontiguousarray(
        xn[0:128].reshape(128, bc * D)).astype(FP8_NP)
    xn1 = np.zeros((128, bc, D), np.float32)
    xn1[0:S - 128] = xn[128:S]
    d["xn1"] = xn1.reshape(128, bc * D).astype(FP8_NP)
    m01 = np.ascontiguousarray(mask.T.astype(np.float32))   # [S, bc]
    d["m01a"] = np.ascontiguousarray(m01[0:128])
    m01b = np.zeros((128, bc), np.float32)
    m01b[0:S - 128] = m01[128:S]
    d["m01b"] = m01b
    d["we2"] = np.concatenate([We, We], 1).astype(BF_NP)
    d["wc2"] = np.concatenate([Wc, Wc], 1).astype(BF_NP)
    d["wp2"] = np.concatenate([Wp, Wp], 1).astype(BF_NP)
    posT = pos.T                                   # [D, S]
    d["pos_ev"] = np.ascontiguousarray(posT[:, 0::2]).astype(BF_NP)
    d["pos_od"] = np.ascontiguousarray(posT[:, 1::2]).astype(BF_NP)
    zzb = np.zeros((128, 512), np.float32)
    for j in range(16):
        zzb[0:64, 32 * j + 2 * j] = z
        zzb[64:128, 32 * j + 2 * j + 1] = z
    d["zzbig"] = zzb.astype(BF_NP)
    d["ones32"] = np.ones((128, 1), np.float32)
    d["ones8"] = np.ones((128, 1), np.float32).astype(FP8_NP)
    d["idf"] = np.eye(128, dtype=np.float32)
    d["xitT"] = np.ascontiguousarray(Xit.T).astype(BF_NP)
    return d


def _unshard(results, bc):
    outs = []
    for k in range(len(results)):
        outs.append(np.concatenate([results[k]["out_attn"],
                                    results[k]["out_ssum"]], axis=1))
    return np.concatenate(outs, axis=0)


def make_in_maps(X_series, pos_series, X_item, valid_mask, Wc, Wp, We, z, bc):
    in_maps = []
    for k in range(N_CORES):
        sl = slice(k * bc, (k + 1) * bc)
        in_maps.append(_prep_core(np.asarray(X_series[sl], np.float32),
                                  np.asarray(X_item[sl], np.float32),
                                  np.asarray(pos_series, np.float32),
                                  np.asarray(valid_mask[sl]),
                                  np.asarray(We, np.float32),
                                  np.asarray(Wp, np.float32),
                                  np.asarray(Wc, np.float32),
                                  np.asarray(z, np.float32), bc))
    return in_maps


def kernel(X_series, pos_series, X_item, valid_mask, Wc, Wp, We, z):
    X_series = np.asarray(X_series, np.float32)
    bc = X_series.shape[0] // N_CORES
    nc = build_nc(bc)
    in_maps = make_in_maps(X_series, pos_series, X_item, valid_mask,
                           Wc, Wp, We, z, bc)
    res = run_bass_kernel_spmd(nc, in_maps, list(range(N_CORES)))
    return _unshard(res.results, bc)


# revision 11
# speedup vs baseline: 1.9104x; 1.1789x over previous
"""Trainium2 Bass kernel for nn_DMRI2INetworkLayer (additive-attention pooling).

Reference (per batch row b):
    pre[s,h]  = X_item[b]@Wc + X_series[b,s]@We + pos[s]@Wp
    scores[s] = sum_h z[h]*tanh(pre[s,h])
    score_sum = sum_s where(mask, scores, 0)
    attn      = softmax(where(mask, scores, -inf))
    out[b]    = concat(sum_s attn[s]*X_series[b,s], score_sum)

Sharding: data-parallel over batch B=4096 across 8 NeuronCores (512 rows
per core). Host does layout/dtype marshalling only; all math on device.

Per-core design (s-major scores phase, [s,b] softmax, fp8 weighted sum):
  - xsm [128d, (s,b)] bf16 streams as moving operand; per s-pair tile the
    two We matmuls (N=512, PE col-groups (0,0)/(0,64)) accumulate onto a
    PSUM bank pre-initialized with the item bias c2 = Wc^T X_item^T (DVE
    copy), so no identity-fold matmuls are needed.
  - tanh on ACT with the pos bias pb[:,t] = [Wp^T pos_even; Wp^T pos_odd]
    as the per-partition activation bias (fused, zero extra cost).
  - z-dot via a block-diagonal stationary zzbig [128, 16*32]: 16 s-pair
    tiles accumulate into one 32-row PSUM group, landing scores directly
    in [s, b] layout (no scatter DMAs, no host permutation).
  - softmax in [s, b]: exp without max-subtraction (|scores| <~ 6 is f32
    safe; bias=-ln16 keeps fp8 attn in range), mask multiply, and
    partition-direction sums (den, score_sum) via ones-stationary matmuls.
  - weighted sum over s: per-b fp8 matmuls X_b^T[s,d] @ attn_b[s,1]
    accumulating into one PSUM bank [128d, 512b]; 1/den folded into the
    final per-partition scale after the PE transpose to [b, d].
  - X ships once per layout: xsm bf16 (26MB) + xn fp8 (13MB) per core.
"""
import os
import sys

sys.path.insert(0, "/opt/trn_rl_repo")

import numpy as np
import ml_dtypes
from contextlib import ExitStack

import concourse.bass as bass
import concourse.bacc as bacc
import concourse.tile as tile
from concourse import mybir
from concourse.bass_utils import run_bass_kernel_spmd

BF = mybir.dt.bfloat16
F32 = mybir.dt.float32
FP8 = mybir.dt.float8e4 if os.environ.get('K_FP8','0')=='1' else mybir.dt.bfloat16
BF_NP = ml_dtypes.bfloat16
FP8_NP = ml_dtypes.float8_e4m3 if os.environ.get('K_FP8','0')=='1' else ml_dtypes.bfloat16

N_CORES = 8
B, S, D, H = 4096, 200, 128, 64
BC = B // N_CORES          # batch rows per core
NT = S // 2                # s-pair tiles (100)
S0 = 128                   # s rows in bank0
S1 = S - S0                # s rows in bank1 (72)
LN16 = float(np.log(16.0))

_CACHE = {}


def build_nc(bc=BC):
    if bc in _CACHE:
        return _CACHE[bc]
    assert bc == 512, "layout hardcoded for bc=512"
    nb = bc // 128             # 128-b output chunks (4)

    nc = bacc.Bacc("TRN2", target_bir_lowering=False, num_devices=N_CORES)

    xsm = nc.declare_dram_parameter("xsm", [D, S * bc], BF, isOutput=False)
    xn0 = nc.declare_dram_parameter("xn0", [128, bc * D], FP8, isOutput=False)
    xn1 = nc.declare_dram_parameter("xn1", [128, bc * D], FP8, isOutput=False)
    m01a = nc.declare_dram_parameter("m01a", [128, bc], F32, isOutput=False)
    m01b = nc.declare_dram_parameter("m01b", [128, bc], F32, isOutput=False)
    we2 = nc.declare_dram_parameter("we2", [D, 128], BF, isOutput=False)
    wc2 = nc.declare_dram_parameter("wc2", [D, 128], BF, isOutput=False)
    wp2 = nc.declare_dram_parameter("wp2", [D, 128], BF, isOutput=False)
    pos_ev = nc.declare_dram_parameter("pos_ev", [D, NT], BF, isOutput=False)
    pos_od = nc.declare_dram_parameter("pos_od", [D, NT], BF, isOutput=False)
    zzbig = nc.declare_dram_parameter("zzbig", [128, 512], BF, isOutput=False)
    ones32 = nc.declare_dram_parameter("ones32", [128, 1], F32, isOutput=False)
    ones8 = nc.declare_dram_parameter("ones8", [128, 1], FP8, isOutput=False)
    idf = nc.declare_dram_parameter("idf", [128, 128], F32, isOutput=False)
    xitT = nc.declare_dram_parameter("xitT", [D, bc], BF, isOutput=False)
    out_attn = nc.declare_dram_parameter("out_attn", [bc, D], F32, isOutput=True)
    out_ssum = nc.declare_dram_parameter("out_ssum", [bc, 1], F32, isOutput=True)

    with tile.TileContext(nc) as tc, ExitStack() as ctx:
        const = ctx.enter_context(tc.tile_pool(name="const", bufs=1))
        xsp = ctx.enter_context(tc.tile_pool(name="xsp", bufs=3))
        thp = ctx.enter_context(tc.tile_pool(name="thp", bufs=4))
        smp = ctx.enter_context(tc.tile_pool(name="smp", bufs=1))
        outp = ctx.enter_context(tc.tile_pool(name="outp", bufs=2))
        xnp = ctx.enter_context(tc.tile_pool(name="xnp", bufs=3))
        pre_ps = ctx.enter_context(tc.tile_pool(name="pre_ps", bufs=3, space="PSUM"))
        sc_ps = ctx.enter_context(tc.tile_pool(name="sc_ps", bufs=1, space="PSUM"))
        o5_ps = ctx.enter_context(tc.tile_pool(name="o5_ps", bufs=1, space="PSUM"))
        t_ps = ctx.enter_context(tc.tile_pool(name="t_ps", bufs=2, space="PSUM"))

        # ---------- constants (scalar DMA queue; sync queue streams xsm) ----
        def cdma(shape, dt_, src, tag):
            t = const.tile(shape, dt_, tag=tag)
            nc.scalar.dma_start(t[:], src)
            return t

        we2_t = cdma([D, 128], BF, we2[:], "we2_t")
        wc2_t = cdma([D, 128], BF, wc2[:], "wc2_t")
        wp2_t = cdma([D, 128], BF, wp2[:], "wp2_t")
        pev_t = cdma([D, NT], BF, pos_ev[:], "pev_t")
        pod_t = cdma([D, NT], BF, pos_od[:], "pod_t")
        zz_t = cdma([128, 512], BF, zzbig[:], "zz_t")
        on32_t = cdma([128, 1], F32, ones32[:], "on32_t")
        on8_t = cdma([128, 1], FP8, ones8[:], "on8_t")
        idf_t = cdma([128, 128], F32, idf[:], "idf_t")
        xitT_t = cdma([D, bc], BF, xitT[:], "xitT_t")
        m01a_t = cdma([128, bc], F32, m01a[:], "m01a_t")
        m01b_t = cdma([128, bc], F32, m01b[:], "m01b_t")

        # ---------- on-chip small precomputes ----------
        # c2[128, bc]: rows 0-63 and 64-127 both = (Wc^T X_item^T)[h, b]
        c2_ps = t_ps.tile([128, bc], F32, tag="tps")
        nc.tensor.matmul(c2_ps[0:64, :], wc2_t[:, 0:64], xitT_t[:],
                         start=True, stop=True, tile_position=(0, 0),
                         skip_group_check=True)
        nc.tensor.matmul(c2_ps[64:128, :], wc2_t[:, 64:128], xitT_t[:],
                         start=True, stop=True, tile_position=(0, 64),
                         skip_group_check=True)
        c2_sb = const.tile([128, bc], F32, tag="c2_sb")
        nc.vector.tensor_copy(c2_sb[:], c2_ps[:])

        # pb[128, NT]: col t = [Wp^T pos[2t]; Wp^T pos[2t+1]]
        pb_ps = t_ps.tile([128, NT], F32, tag="tps")
        nc.tensor.matmul(pb_ps[0:64, :], wp2_t[:, 0:64], pev_t[:],
                         start=True, stop=True, tile_position=(0, 0),
                         skip_group_check=True)
        nc.tensor.matmul(pb_ps[64:128, :], wp2_t[:, 64:128], pod_t[:],
                         start=True, stop=True, tile_position=(0, 64),
                         skip_group_check=True)
        pb_sb = const.tile([128, NT], F32, tag="pb_sb")
        nc.vector.tensor_copy(pb_sb[:], pb_ps[:])

        # persistent PSUM: scores banks, weighted-sum accumulator
        sc0 = sc_ps.tile([128, bc], F32, tag="sc0")
        sc1 = sc_ps.tile([128, bc], F32, tag="sc1")
        nc.vector.memset(sc1[96:128, :], 0.0)   # rows never written by MMs
        o5 = o5_ps.tile([D, bc], F32, tag="o5")
        nc.vector.memset(o5[:], 0.0)
        dsum_sb = const.tile([128, bc], F32, tag="dsum_sb")
        nc.vector.memset(dsum_sb[:], 0.0)
        expb = const.tile([128, 1], F32, tag="expb")
        nc.vector.memset(expb[:], -LN16)

        # ---------- phase 1: scores in [s, b] ----------
        CH = 4                       # s-pair tiles per DMA chunk
        for chunk in range(NT // CH):
            xt = xsp.tile([128, CH * 2 * bc], BF, tag="xt")
            nc.sync.dma_start(xt[:], xsm[:, chunk * CH * 2 * bc:
                                         (chunk + 1) * CH * 2 * bc])
            for j in range(CH):
                t = chunk * CH + j
                xe = xt[:, (2 * j) * bc:(2 * j + 1) * bc]
                xo = xt[:, (2 * j + 1) * bc:(2 * j + 2) * bc]
                pre = pre_ps.tile([128, bc], F32, tag="pre")
                nc.vector.tensor_copy(pre[:], c2_sb[:])
                nc.tensor.matmul(pre[0:64, :], we2_t[:, 0:64], xe,
                                 start=False, stop=False, tile_position=(0, 0),
                                 skip_group_check=True)
                nc.tensor.matmul(pre[64:128, :], we2_t[:, 64:128], xo,
                                 start=False, stop=True, tile_position=(0, 64),
                                 skip_group_check=True)
                th = thp.tile([128, bc], BF, tag="th")
                nc.scalar.activation(th[:], pre[:],
                                     mybir.ActivationFunctionType.Tanh,
                                     bias=pb_sb[:, t:t + 1])
                g, jj = divmod(t, 16)
                tgt, ro = (sc0, 32 * g) if g < 4 else (sc1, 32 * (g - 4))
                nc.tensor.matmul(tgt[ro:ro + 32, :],
                                 zz_t[:, 32 * jj:32 * jj + 32], th[:],
                                 start=(jj == 0), stop=(jj == 15 or t == NT - 1),
                                 tile_position=(0, ro), skip_group_check=True)

        # ---------- phase 2: masked softmax pieces in [s, b] ----------
        ms0 = smp.tile([128, bc], F32, tag="ms0")
        nc.vector.tensor_mul(ms0[:], sc0[:], m01a_t[:])
        ms1 = smp.tile([128, bc], F32, tag="ms1")
        nc.vector.tensor_mul(ms1[:], sc1[:], m01b_t[:])
        e0 = smp.tile([128, bc], F32, tag="e0")
        nc.scalar.activation(e0[:], sc0[:], mybir.ActivationFunctionType.Exp,
                             bias=expb[:])
        e1 = smp.tile([128, bc], F32, tag="e1")
        nc.scalar.activation(e1[:], sc1[:], mybir.ActivationFunctionType.Exp,
                             bias=expb[:])
        att0 = smp.tile([128, bc], FP8, tag="att0")
        nc.vector.tensor_mul(att0[:], e0[:], m01a_t[:])
        att1 = smp.tile([128, bc], FP8, tag="att1")
        nc.vector.tensor_mul(att1[:], e1[:], m01b_t[:])

        dsum = t_ps.tile([128, bc], F32, tag="tps")
        nc.tensor.matmul(dsum[0:1, :], on8_t[:], att0[:],
                         start=True, stop=False, tile_position=(0, 0),
                         skip_group_check=True)
        nc.tensor.matmul(dsum[0:1, :], on8_t[:], att1[:],
                         start=False, stop=True, tile_position=(0, 0),
                         skip_group_check=True)
        nc.tensor.matmul(dsum[32:33, :], on32_t[:], ms0[:],
                         start=True, stop=False, tile_position=(0, 32),
                         skip_group_check=True)
        nc.tensor.matmul(dsum[32:33, :], on32_t[:], ms1[:],
                         start=False, stop=True, tile_position=(0, 32),
                         skip_group_check=True)
        nc.vector.tensor_copy(dsum_sb[0:1, :], dsum[0:1, :])
        nc.vector.tensor_copy(dsum_sb[32:33, :], dsum[32:33, :])

        rdens = []
        for c in range(nb):
            dt_ps = t_ps.tile([128, 128], F32, tag="tps")
            nc.tensor.transpose(dt_ps[:], dsum_sb[:, c * 128:(c + 1) * 128],
                                idf_t[:])
            rden = smp.tile([128, 1], F32, tag="rden", bufs=4)
            nc.vector.reciprocal(rden[:], dt_ps[:, 0:1])
            rdens.append(rden)
            ssc = smp.tile([128, 1], F32, tag="ssc", bufs=2)
            nc.vector.tensor_copy(ssc[:], dt_ps[:, 32:33])
            nc.sync.dma_start(out_ssum[c * 128:(c + 1) * 128, :], ssc[:])

        # ---------- phase 3: weighted sum over s ----------
        n5 = 0
        XB = 64
        for ch in range(bc // XB):
            xn0_c = xnp.tile([128, XB * D], FP8, tag="xn0_c")
            nc.gpsimd.dma_start(xn0_c[:], xn0[:, ch * XB * D:(ch + 1) * XB * D])
            xn1_c = xnp.tile([128, XB * D], FP8, tag="xn1_c")
            nc.gpsimd.dma_start(xn1_c[:], xn1[:, ch * XB * D:(ch + 1) * XB * D])
            for i in range(XB):
                b = ch * XB + i
                n5 += 2
                nc.tensor.matmul(o5[:, b:b + 1], xn0_c[:, i * D:(i + 1) * D],
                                 att0[:, b:b + 1], start=False, stop=False,
                                 skip_group_check=True)
                nc.tensor.matmul(o5[:, b:b + 1], xn1_c[:, i * D:(i + 1) * D],
                                 att1[:, b:b + 1], start=False, stop=(n5 == 2 * bc),
                                 skip_group_check=True)

        # ---------- output: transpose [d, b] -> [b, d], scale by 1/den ----
        o5_s = outp.tile([D, bc], F32, tag="o5_s")
        nc.vector.tensor_copy(o5_s[:], o5[:])
        for c in range(nb):
            ot_ps = t_ps.tile([128, 128], F32, tag="tps")
            nc.tensor.transpose(ot_ps[:], o5_s[:, c * 128:(c + 1) * 128],
                                idf_t[:])
            ob = outp.tile([128, D], F32, tag="ob")
            nc.vector.tensor_scalar_mul(ob[:], ot_ps[:], rdens[c][:])
            nc.sync.dma_start(out_attn[c * 128:(c + 1) * 128, :], ob[:])

    nc.compile()
    _CACHE[bc] = nc
    return nc


def _prep_core(Xs, Xit, pos, mask, We, Wp, Wc, z, bc):
    """Host-side marshalling (layout/dtype only) for one core's shard."""
    d = {}
    d["xsm"] = np.ascontiguousarray(
        Xs.transpose(2, 1, 0).reshape(D, S * bc)).astype(BF_NP)
    xn = Xs.transpose(1, 0, 2)                     # [S, bc, D]
    d["xn0"] = np.ascontiguousarray(
        xn[0:128].reshape(128, bc * D)).astype(FP8_NP)
    xn1 = np.zeros((128, bc, D), np.float32)
    xn1[0:S - 128] = xn[128:S]
    d["xn1"] = xn1.reshape(128, bc * D).astype(FP8_NP)
    m01 = np.ascontiguousarray(mask.T.astype(np.float32))   # [S, bc]
    d["m01a"] = np.ascontiguousarray(m01[0:128])
    m01b = np.zeros((128, bc), np.float32)
    m01b[0:S - 128] = m01[128:S]
    d["m01b"] = m01b
    d["we2"] = np.concatenate([We, We], 1).astype(BF_NP)
    d["wc2"] = np.concatenate([Wc, Wc], 1).astype(BF_NP)
    d["wp2"] = np.concatenate([Wp, Wp], 1).astype(BF_NP)
    posT = pos.T                                   # [D, S]
    d["pos_ev"] = np.ascontiguousarray(posT[:, 0::2]).astype(BF_NP)
    d["pos_od"] = np.ascontiguousarray(posT[:, 1::2]).astype(BF_NP)
    zzb = np.zeros((128, 512), np.float32)
    for j in range(16):
        zzb[0:64, 32 * j + 2 * j] = z
        zzb[64:128, 32 * j + 2 * j + 1] = z
    d["zzbig"] = zzb.astype(BF_NP)
    d["ones32"] = np.ones((128, 1), np.float32)
    d["ones8"] = np.ones((128, 1), np.float32).astype(FP8_NP)
    d["idf"] = np.eye(128, dtype=np.float32)
    d["xitT"] = np.ascontiguousarray(Xit.T).astype(BF_NP)
    return d


def _unshard(results, bc):
    outs = []
    for k in range(len(results)):
        outs.append(np.concatenate([results[k]["out_attn"],
                                    results[k]["out_ssum"]], axis=1))
    return np.concatenate(outs, axis=0)


def make_in_maps(X_series, pos_series, X_item, valid_mask, Wc, Wp, We, z, bc):
    in_maps = []
    for k in range(N_CORES):
        sl = slice(k * bc, (k + 1) * bc)
        in_maps.append(_prep_core(np.asarray(X_series[sl], np.float32),
                                  np.asarray(X_item[sl], np.float32),
                                  np.asarray(pos_series, np.float32),
                                  np.asarray(valid_mask[sl]),
                                  np.asarray(We, np.float32),
                                  np.asarray(Wp, np.float32),
                                  np.asarray(Wc, np.float32),
                                  np.asarray(z, np.float32), bc))
    return in_maps


def kernel(X_series, pos_series, X_item, valid_mask, Wc, Wp, We, z):
    X_series = np.asarray(X_series, np.float32)
    bc = X_series.shape[0] // N_CORES
    nc = build_nc(bc)
    in_maps = make_in_maps(X_series, pos_series, X_item, valid_mask,
                           Wc, Wp, We, z, bc)
    res = run_bass_kernel_spmd(nc, in_maps, list(range(N_CORES)))
    return _unshard(res.results, bc)


# revision 12
# speedup vs baseline: 2.0989x; 1.0987x over previous
"""Trainium2 Bass kernel for nn_DMRI2INetworkLayer (additive-attention pooling).

Reference (per batch row b):
    pre[s,h]  = X_item[b]@Wc + X_series[b,s]@We + pos[s]@Wp
    scores[s] = sum_h z[h]*tanh(pre[s,h])
    score_sum = sum_s where(mask, scores, 0)
    attn      = softmax(where(mask, scores, -inf))
    out[b]    = concat(sum_s attn[s]*X_series[b,s], score_sum)

Sharding: data-parallel over batch B=4096 across 8 NeuronCores (512 rows
per core). Host does layout/dtype marshalling only; all math on device.

Per-core design (s-major scores phase, [s,b] softmax, fp8 weighted sum):
  - xsm [128d, (s,b)] bf16 streams as moving operand; per s-pair tile the
    two We matmuls (N=512, PE col-groups (0,0)/(0,64)) accumulate onto a
    PSUM bank pre-initialized with the item bias c2 = Wc^T X_item^T (DVE
    copy), so no identity-fold matmuls are needed.
  - tanh on ACT with the pos bias pb[:,t] = [Wp^T pos_even; Wp^T pos_odd]
    as the per-partition activation bias (fused, zero extra cost).
  - z-dot via a block-diagonal stationary zzbig [128, 16*32]: 16 s-pair
    tiles accumulate into one 32-row PSUM group, landing scores directly
    in [s, b] layout (no scatter DMAs, no host permutation).
  - softmax in [s, b]: exp without max-subtraction (|scores| <~ 6 is f32
    safe; bias=-ln16 keeps fp8 attn in range), mask multiply, and
    partition-direction sums (den, score_sum) via ones-stationary matmuls.
  - weighted sum over s: per-b fp8 matmuls X_b^T[s,d] @ attn_b[s,1]
    accumulating into one PSUM bank [128d, 512b]; 1/den folded into the
    final per-partition scale after the PE transpose to [b, d].
  - X ships once per layout: xsm bf16 (26MB) + xn fp8 (13MB) per core.
"""
import os
import sys

sys.path.insert(0, "/opt/trn_rl_repo")

import numpy as np
import ml_dtypes
from contextlib import ExitStack

import concourse.bass as bass
import concourse.bacc as bacc
import concourse.tile as tile
from concourse import mybir
from concourse.bass_utils import run_bass_kernel_spmd

BF = mybir.dt.bfloat16
F32 = mybir.dt.float32
FP8 = mybir.dt.float8e4 if os.environ.get('K_FP8','0')=='1' else mybir.dt.bfloat16
BF_NP = ml_dtypes.bfloat16
FP8_NP = ml_dtypes.float8_e4m3 if os.environ.get('K_FP8','0')=='1' else ml_dtypes.bfloat16

N_CORES = 8
B, S, D, H = 4096, 200, 128, 64
BC = B // N_CORES          # batch rows per core
NT = S // 2                # s-pair tiles (100)
S0 = 128                   # s rows in bank0
S1 = S - S0                # s rows in bank1 (72)
LN16 = float(np.log(16.0))

_CACHE = {}


def build_nc(bc=BC):
    if bc in _CACHE:
        return _CACHE[bc]
    assert bc == 512, "layout hardcoded for bc=512"
    nb = bc // 128             # 128-b output chunks (4)

    nc = bacc.Bacc("TRN2", target_bir_lowering=False, num_devices=N_CORES)

    xsm = nc.declare_dram_parameter("xsm", [D, S * bc], BF, isOutput=False)
    xn0 = nc.declare_dram_parameter("xn0", [128, bc * D], FP8, isOutput=False)
    xn1 = nc.declare_dram_parameter("xn1", [128, bc * D], FP8, isOutput=False)
    m01a = nc.declare_dram_parameter("m01a", [128, bc], F32, isOutput=False)
    m01b = nc.declare_dram_parameter("m01b", [128, bc], F32, isOutput=False)
    we2 = nc.declare_dram_parameter("we2", [D, 128], BF, isOutput=False)
    wc2 = nc.declare_dram_parameter("wc2", [D, 128], BF, isOutput=False)
    wp2 = nc.declare_dram_parameter("wp2", [D, 128], BF, isOutput=False)
    pos_ev = nc.declare_dram_parameter("pos_ev", [D, NT], BF, isOutput=False)
    pos_od = nc.declare_dram_parameter("pos_od", [D, NT], BF, isOutput=False)
    zzbig = nc.declare_dram_parameter("zzbig", [128, 512], BF, isOutput=False)
    ones32 = nc.declare_dram_parameter("ones32", [128, 1], F32, isOutput=False)
    ones8 = nc.declare_dram_parameter("ones8", [128, 1], FP8, isOutput=False)
    idf = nc.declare_dram_parameter("idf", [128, 128], F32, isOutput=False)
    xitT = nc.declare_dram_parameter("xitT", [D, bc], BF, isOutput=False)
    out_attn = nc.declare_dram_parameter("out_attn", [bc, D], F32, isOutput=True)
    out_ssum = nc.declare_dram_parameter("out_ssum", [bc, 1], F32, isOutput=True)

    with tile.TileContext(nc) as tc, ExitStack() as ctx:
        const = ctx.enter_context(tc.tile_pool(name="const", bufs=1))
        xsp = ctx.enter_context(tc.tile_pool(name="xsp", bufs=3))
        thp = ctx.enter_context(tc.tile_pool(name="thp", bufs=4))
        smp = ctx.enter_context(tc.tile_pool(name="smp", bufs=1))
        outp = ctx.enter_context(tc.tile_pool(name="outp", bufs=2))
        xnp = ctx.enter_context(tc.tile_pool(name="xnp", bufs=3))
        pre_ps = ctx.enter_context(tc.tile_pool(name="pre_ps", bufs=3, space="PSUM"))
        sc_ps = ctx.enter_context(tc.tile_pool(name="sc_ps", bufs=1, space="PSUM"))
        o5_ps = ctx.enter_context(tc.tile_pool(name="o5_ps", bufs=1, space="PSUM"))
        t_ps = ctx.enter_context(tc.tile_pool(name="t_ps", bufs=2, space="PSUM"))

        # ---------- constants (scalar DMA queue; sync queue streams xsm) ----
        def cdma(shape, dt_, src, tag):
            t = const.tile(shape, dt_, tag=tag)
            nc.scalar.dma_start(t[:], src)
            return t

        we2_t = cdma([D, 128], BF, we2[:], "we2_t")
        wc2_t = cdma([D, 128], BF, wc2[:], "wc2_t")
        wp2_t = cdma([D, 128], BF, wp2[:], "wp2_t")
        pev_t = cdma([D, NT], BF, pos_ev[:], "pev_t")
        pod_t = cdma([D, NT], BF, pos_od[:], "pod_t")
        zz_t = cdma([128, 512], BF, zzbig[:], "zz_t")
        on32_t = cdma([128, 1], F32, ones32[:], "on32_t")
        on8_t = cdma([128, 1], FP8, ones8[:], "on8_t")
        idf_t = cdma([128, 128], F32, idf[:], "idf_t")
        xitT_t = cdma([D, bc], BF, xitT[:], "xitT_t")
        m01a_t = cdma([128, bc], F32, m01a[:], "m01a_t")
        m01b_t = cdma([128, bc], F32, m01b[:], "m01b_t")

        # ---------- on-chip small precomputes ----------
        # c2[128, bc]: rows 0-63 and 64-127 both = (Wc^T X_item^T)[h, b]
        c2_ps = t_ps.tile([128, bc], F32, tag="tps")
        nc.tensor.matmul(c2_ps[0:64, :], wc2_t[:, 0:64], xitT_t[:],
                         start=True, stop=True, tile_position=(0, 0),
                         skip_group_check=True)
        nc.tensor.matmul(c2_ps[64:128, :], wc2_t[:, 64:128], xitT_t[:],
                         start=True, stop=True, tile_position=(0, 64),
                         skip_group_check=True)
        c2_sb = const.tile([128, bc], F32, tag="c2_sb")
        nc.vector.tensor_copy(c2_sb[:], c2_ps[:])

        # pb[128, NT]: col t = [Wp^T pos[2t]; Wp^T pos[2t+1]]
        pb_ps = t_ps.tile([128, NT], F32, tag="tps")
        nc.tensor.matmul(pb_ps[0:64, :], wp2_t[:, 0:64], pev_t[:],
                         start=True, stop=True, tile_position=(0, 0),
                         skip_group_check=True)
        nc.tensor.matmul(pb_ps[64:128, :], wp2_t[:, 64:128], pod_t[:],
                         start=True, stop=True, tile_position=(0, 64),
                         skip_group_check=True)
        pb_sb = const.tile([128, NT], F32, tag="pb_sb")
        nc.vector.tensor_copy(pb_sb[:], pb_ps[:])

        # persistent PSUM: scores banks, weighted-sum accumulator
        sc0 = sc_ps.tile([128, bc], F32, tag="sc0")
        sc1 = sc_ps.tile([128, bc], F32, tag="sc1")
        nc.vector.memset(sc1[96:128, :], 0.0)   # rows never written by MMs
        o5 = o5_ps.tile([D, bc], F32, tag="o5")
        nc.vector.memset(o5[:], 0.0)
        dsum_sb = const.tile([128, bc], F32, tag="dsum_sb")
        nc.vector.memset(dsum_sb[:], 0.0)
        expb = const.tile([128, 1], F32, tag="expb")
        nc.vector.memset(expb[:], -LN16)

        # ---------- phase 1: scores in [s, b] ----------
        CH = 4                       # s-pair tiles per DMA chunk
        for chunk in range(NT // CH):
            xt = xsp.tile([128, CH * 2 * bc], BF, tag="xt")
            nc.sync.dma_start(xt[:], xsm[:, chunk * CH * 2 * bc:
                                         (chunk + 1) * CH * 2 * bc])
            for j in range(CH):
                t = chunk * CH + j
                xe = xt[:, (2 * j) * bc:(2 * j + 1) * bc]
                xo = xt[:, (2 * j + 1) * bc:(2 * j + 2) * bc]
                pre = pre_ps.tile([128, bc], F32, tag="pre")
                nc.vector.tensor_copy(pre[:], c2_sb[:])
                nc.tensor.matmul(pre[0:64, :], we2_t[:, 0:64], xe,
                                 start=False, stop=False, tile_position=(0, 0),
                                 skip_group_check=True)
                nc.tensor.matmul(pre[64:128, :], we2_t[:, 64:128], xo,
                                 start=False, stop=True, tile_position=(0, 64),
                                 skip_group_check=True)
                th = thp.tile([128, bc], BF, tag="th")
                nc.scalar.activation(th[:], pre[:],
                                     mybir.ActivationFunctionType.Tanh,
                                     bias=pb_sb[:, t:t + 1])
                g, jj = divmod(t, 16)
                tgt, ro = (sc0, 32 * g) if g < 4 else (sc1, 32 * (g - 4))
                nc.tensor.matmul(tgt[ro:ro + 32, :],
                                 zz_t[:, 32 * jj:32 * jj + 32], th[:],
                                 start=(jj == 0), stop=(jj == 15 or t == NT - 1),
                                 tile_position=(0, ro), skip_group_check=True)

        # ---------- phase 2: masked softmax pieces in [s, b] ----------
        ms0 = smp.tile([128, bc], F32, tag="ms0")
        nc.vector.tensor_mul(ms0[:], sc0[:], m01a_t[:])
        ms1 = smp.tile([128, bc], F32, tag="ms1")
        nc.vector.tensor_mul(ms1[:], sc1[:], m01b_t[:])
        e0 = smp.tile([128, bc], F32, tag="e0")
        nc.scalar.activation(e0[:], sc0[:], mybir.ActivationFunctionType.Exp,
                             bias=expb[:])
        e1 = smp.tile([128, bc], F32, tag="e1")
        nc.scalar.activation(e1[:], sc1[:], mybir.ActivationFunctionType.Exp,
                             bias=expb[:])
        att0 = smp.tile([128, bc], FP8, tag="att0")
        nc.vector.tensor_mul(att0[:], e0[:], m01a_t[:])
        att1 = smp.tile([128, bc], FP8, tag="att1")
        nc.vector.tensor_mul(att1[:], e1[:], m01b_t[:])

        dsum = t_ps.tile([128, bc], F32, tag="tps")
        nc.tensor.matmul(dsum[0:1, :], on8_t[:], att0[:],
                         start=True, stop=False, tile_position=(0, 0),
                         skip_group_check=True)
        nc.tensor.matmul(dsum[0:1, :], on8_t[:], att1[:],
                         start=False, stop=True, tile_position=(0, 0),
                         skip_group_check=True)
        nc.tensor.matmul(dsum[32:33, :], on32_t[:], ms0[:],
                         start=True, stop=False, tile_position=(0, 32),
                         skip_group_check=True)
        nc.tensor.matmul(dsum[32:33, :], on32_t[:], ms1[:],
                         start=False, stop=True, tile_position=(0, 32),
                         skip_group_check=True)
        nc.vector.tensor_copy(dsum_sb[0:1, :], dsum[0:1, :])
        nc.vector.tensor_copy(dsum_sb[32:33, :], dsum[32:33, :])

        rdens = []
        for c in range(nb):
            dt_ps = t_ps.tile([128, 128], F32, tag="tps")
            nc.tensor.transpose(dt_ps[:], dsum_sb[:, c * 128:(c + 1) * 128],
                                idf_t[:])
            rden = smp.tile([128, 1], F32, tag="rden", bufs=4)
            nc.vector.reciprocal(rden[:], dt_ps[:, 0:1])
            rdens.append(rden)
            ssc = smp.tile([128, 1], F32, tag="ssc", bufs=2)
            nc.vector.tensor_copy(ssc[:], dt_ps[:, 32:33])
            nc.sync.dma_start(out_ssum[c * 128:(c + 1) * 128, :], ssc[:])

        # ---------- phase 3: weighted sum over s ----------
        n5 = 0
        XB = 64
        for ch in range(bc // XB):
            xn0_c = xnp.tile([128, XB * D], FP8, tag="xn0_c")
            nc.scalar.dma_start(xn0_c[:], xn0[:, ch * XB * D:(ch + 1) * XB * D])
            xn1_c = xnp.tile([128, XB * D], FP8, tag="xn1_c")
            nc.scalar.dma_start(xn1_c[:], xn1[:, ch * XB * D:(ch + 1) * XB * D])
            for i in range(XB):
                b = ch * XB + i
                n5 += 2
                nc.tensor.matmul(o5[:, b:b + 1], xn0_c[:, i * D:(i + 1) * D],
                                 att0[:, b:b + 1], start=False, stop=False,
                                 skip_group_check=True)
                nc.tensor.matmul(o5[:, b:b + 1], xn1_c[:, i * D:(i + 1) * D],
                                 att1[:, b:b + 1], start=False, stop=(n5 == 2 * bc),
                                 skip_group_check=True)

        # ---------- output: transpose [d, b] -> [b, d], scale by 1/den ----
        o5_s = outp.tile([D, bc], F32, tag="o5_s")
        nc.vector.tensor_copy(o5_s[:], o5[:])
        for c in range(nb):
            ot_ps = t_ps.tile([128, 128], F32, tag="tps")
            nc.tensor.transpose(ot_ps[:], o5_s[:, c * 128:(c + 1) * 128],
                                idf_t[:])
            ob = outp.tile([128, D], F32, tag="ob")
            nc.vector.tensor_scalar_mul(ob[:], ot_ps[:], rdens[c][:])
            nc.sync.dma_start(out_attn[c * 128:(c + 1) * 128, :], ob[:])

    nc.compile()
    _CACHE[bc] = nc
    return nc


def _prep_core(Xs, Xit, pos, mask, We, Wp, Wc, z, bc):
    """Host-side marshalling (layout/dtype only) for one core's shard."""
    d = {}
    d["xsm"] = np.ascontiguousarray(
        Xs.transpose(2, 1, 0).reshape(D, S * bc)).astype(BF_NP)
    xn = Xs.transpose(1, 0, 2)                     # [S, bc, D]
    d["xn0"] = np.ascontiguousarray(
        xn[0:128].reshape(128, bc * D)).astype(FP8_NP)
    xn1 = np.zeros((128, bc, D), np.float32)
    xn1[0:S - 128] = xn[128:S]
    d["xn1"] = xn1.reshape(128, bc * D).astype(FP8_NP)
    m01 = np.ascontiguousarray(mask.T.astype(np.float32))   # [S, bc]
    d["m01a"] = np.ascontiguousarray(m01[0:128])
    m01b = np.zeros((128, bc), np.float32)
    m01b[0:S - 128] = m01[128:S]
    d["m01b"] = m01b
    d["we2"] = np.concatenate([We, We], 1).astype(BF_NP)
    d["wc2"] = np.concatenate([Wc, Wc], 1).astype(BF_NP)
    d["wp2"] = np.concatenate([Wp, Wp], 1).astype(BF_NP)
    posT = pos.T                                   # [D, S]
    d["pos_ev"] = np.ascontiguousarray(posT[:, 0::2]).astype(BF_NP)
    d["pos_od"] = np.ascontiguousarray(posT[:, 1::2]).astype(BF_NP)
    zzb = np.zeros((128, 512), np.float32)
    for j in range(16):
        zzb[0:64, 32 * j + 2 * j] = z
        zzb[64:128, 32 * j + 2 * j + 1] = z
    d["zzbig"] = zzb.astype(BF_NP)
    d["ones32"] = np.ones((128, 1), np.float32)
    d["ones8"] = np.ones((128, 1), np.float32).astype(FP8_NP)
    d["idf"] = np.eye(128, dtype=np.float32)
    d["xitT"] = np.ascontiguousarray(Xit.T).astype(BF_NP)
    return d


def _unshard(results, bc):
    outs = []
    for k in range(len(results)):
        outs.append(np.concatenate([results[k]["out_attn"],
                                    results[k]["out_ssum"]], axis=1))
    return np.concatenate(outs, axis=0)


def make_in_maps(X_series, pos_series, X_item, valid_mask, Wc, Wp, We, z, bc):
    in_maps = []
    for k in range(N_CORES):
        sl = slice(k * bc, (k + 1) * bc)
        in_maps.append(_prep_core(np.asarray(X_series[sl], np.float32),
                                  np.asarray(X_item[sl], np.float32),
                                  np.asarray(pos_series, np.float32),
                                  np.asarray(valid_mask[sl]),
                                  np.asarray(We, np.float32),
                                  np.asarray(Wp, np.float32),
                                  np.asarray(Wc, np.float32),
                                  np.asarray(z, np.float32), bc))
    return in_maps


def kernel(X_series, pos_series, X_item, valid_mask, Wc, Wp, We, z):
    X_series = np.asarray(X_series, np.float32)
    bc = X_series.shape[0] // N_CORES
    nc = build_nc(bc)
    in_maps = make_in_maps(X_series, pos_series, X_item, valid_mask,
                           Wc, Wp, We, z, bc)
    res = run_bass_kernel_spmd(nc, in_maps, list(range(N_CORES)))
    return _unshard(res.results, bc)


# revision 13
# speedup vs baseline: 2.1578x; 1.0281x over previous
"""Trainium2 Bass kernel for nn_DMRI2INetworkLayer (additive-attention pooling).

Reference (per batch row b):
    pre[s,h]  = X_item[b]@Wc + X_series[b,s]@We + pos[s]@Wp
    scores[s] = sum_h z[h]*tanh(pre[s,h])
    score_sum = sum_s where(mask, scores, 0)
    attn      = softmax(where(mask, scores, -inf))
    out[b]    = concat(sum_s attn[s]*X_series[b,s], score_sum)

Sharding: data-parallel over batch B=4096 across 8 NeuronCores (512 rows
per core). Host does layout/dtype marshalling only; all math on device.

Per-core design (s-major scores phase, [s,b] softmax, fp8 weighted sum):
  - xsm [128d, (s,b)] bf16 streams as moving operand; per s-pair tile the
    two We matmuls (N=512, PE col-groups (0,0)/(0,64)) accumulate onto a
    PSUM bank pre-initialized with the item bias c2 = Wc^T X_item^T (DVE
    copy), so no identity-fold matmuls are needed.
  - tanh on ACT with the pos bias pb[:,t] = [Wp^T pos_even; Wp^T pos_odd]
    as the per-partition activation bias (fused, zero extra cost).
  - z-dot via a block-diagonal stationary zzbig [128, 16*32]: 16 s-pair
    tiles accumulate into one 32-row PSUM group, landing scores directly
    in [s, b] layout (no scatter DMAs, no host permutation).
  - softmax in [s, b]: exp without max-subtraction (|scores| <~ 6 is f32
    safe; bias=-ln16 keeps fp8 attn in range), mask multiply, and
    partition-direction sums (den, score_sum) via ones-stationary matmuls.
  - weighted sum over s: per-b fp8 matmuls X_b^T[s,d] @ attn_b[s,1]
    accumulating into one PSUM bank [128d, 512b]; 1/den folded into the
    final per-partition scale after the PE transpose to [b, d].
  - X ships once per layout: xsm bf16 (26MB) + xn fp8 (13MB) per core.
"""
import os
import sys

sys.path.insert(0, "/opt/trn_rl_repo")

import numpy as np
import ml_dtypes
from contextlib import ExitStack

import concourse.bass as bass
import concourse.bacc as bacc
import concourse.tile as tile
from concourse import mybir
from concourse.bass_utils import run_bass_kernel_spmd

BF = mybir.dt.bfloat16
F32 = mybir.dt.float32
FP8 = mybir.dt.float8e4 if os.environ.get('K_FP8','0')=='1' else mybir.dt.bfloat16
BF_NP = ml_dtypes.bfloat16
FP8_NP = ml_dtypes.float8_e4m3 if os.environ.get('K_FP8','0')=='1' else ml_dtypes.bfloat16

N_CORES = 8
B, S, D, H = 4096, 200, 128, 64
BC = B // N_CORES          # batch rows per core
NT = S // 2                # s-pair tiles (100)
S0 = 128                   # s rows in bank0
S1 = S - S0                # s rows in bank1 (72)
LN16 = float(np.log(16.0))

_CACHE = {}


def build_nc(bc=BC):
    if bc in _CACHE:
        return _CACHE[bc]
    assert bc == 512, "layout hardcoded for bc=512"
    nb = bc // 128             # 128-b output chunks (4)

    nc = bacc.Bacc("TRN2", target_bir_lowering=False, num_devices=N_CORES)

    xsm = nc.declare_dram_parameter("xsm", [D, S * bc], BF, isOutput=False)
    xn0 = nc.declare_dram_parameter("xn0", [128, bc * D], FP8, isOutput=False)
    xn1 = nc.declare_dram_parameter("xn1", [128, bc * D], FP8, isOutput=False)
    m01a = nc.declare_dram_parameter("m01a", [128, bc], F32, isOutput=False)
    m01b = nc.declare_dram_parameter("m01b", [128, bc], F32, isOutput=False)
    we2 = nc.declare_dram_parameter("we2", [D, 128], BF, isOutput=False)
    wc2 = nc.declare_dram_parameter("wc2", [D, 128], BF, isOutput=False)
    wp2 = nc.declare_dram_parameter("wp2", [D, 128], BF, isOutput=False)
    pos_ev = nc.declare_dram_parameter("pos_ev", [D, NT], BF, isOutput=False)
    pos_od = nc.declare_dram_parameter("pos_od", [D, NT], BF, isOutput=False)
    zzbig = nc.declare_dram_parameter("zzbig", [128, 512], BF, isOutput=False)
    ones32 = nc.declare_dram_parameter("ones32", [128, 1], F32, isOutput=False)
    ones8 = nc.declare_dram_parameter("ones8", [128, 1], FP8, isOutput=False)
    idf = nc.declare_dram_parameter("idf", [128, 128], F32, isOutput=False)
    xitT = nc.declare_dram_parameter("xitT", [D, bc], BF, isOutput=False)
    out_attn = nc.declare_dram_parameter("out_attn", [bc, D], F32, isOutput=True)
    out_ssum = nc.declare_dram_parameter("out_ssum", [bc, 1], F32, isOutput=True)

    with tile.TileContext(nc) as tc, ExitStack() as ctx:
        const = ctx.enter_context(tc.tile_pool(name="const", bufs=1))
        xsp = ctx.enter_context(tc.tile_pool(name="xsp", bufs=3))
        thp = ctx.enter_context(tc.tile_pool(name="thp", bufs=4))
        smp = ctx.enter_context(tc.tile_pool(name="smp", bufs=1))
        outp = ctx.enter_context(tc.tile_pool(name="outp", bufs=2))
        xnp = ctx.enter_context(tc.tile_pool(name="xnp", bufs=6))
        pre_ps = ctx.enter_context(tc.tile_pool(name="pre_ps", bufs=3, space="PSUM"))
        sc_ps = ctx.enter_context(tc.tile_pool(name="sc_ps", bufs=1, space="PSUM"))
        o5_ps = ctx.enter_context(tc.tile_pool(name="o5_ps", bufs=1, space="PSUM"))
        t_ps = ctx.enter_context(tc.tile_pool(name="t_ps", bufs=2, space="PSUM"))

        # ---------- constants (scalar DMA queue; sync queue streams xsm) ----
        def cdma(shape, dt_, src, tag):
            t = const.tile(shape, dt_, tag=tag)
            nc.scalar.dma_start(t[:], src)
            return t

        wc2_t = cdma([D, 128], BF, wc2[:], "wc2_t")
        xitT_t = cdma([D, bc], BF, xitT[:], "xitT_t")
        we2_t = cdma([D, 128], BF, we2[:], "we2_t")
        wp2_t = cdma([D, 128], BF, wp2[:], "wp2_t")
        pev_t = cdma([D, NT], BF, pos_ev[:], "pev_t")
        pod_t = cdma([D, NT], BF, pos_od[:], "pod_t")
        zz_t = cdma([128, 512], BF, zzbig[:], "zz_t")
        on32_t = cdma([128, 1], F32, ones32[:], "on32_t")
        on8_t = cdma([128, 1], FP8, ones8[:], "on8_t")
        idf_t = cdma([128, 128], F32, idf[:], "idf_t")
        m01a_t = cdma([128, bc], F32, m01a[:], "m01a_t")
        m01b_t = cdma([128, bc], F32, m01b[:], "m01b_t")

        # ---------- on-chip small precomputes ----------
        # c2[128, bc]: rows 0-63 and 64-127 both = (Wc^T X_item^T)[h, b]
        c2_ps = t_ps.tile([128, bc], F32, tag="tps")
        nc.tensor.matmul(c2_ps[0:64, :], wc2_t[:, 0:64], xitT_t[:],
                         start=True, stop=True, tile_position=(0, 0),
                         skip_group_check=True)
        nc.tensor.matmul(c2_ps[64:128, :], wc2_t[:, 64:128], xitT_t[:],
                         start=True, stop=True, tile_position=(0, 64),
                         skip_group_check=True)
        c2_sb = const.tile([128, bc], F32, tag="c2_sb")
        nc.vector.tensor_copy(c2_sb[:], c2_ps[:])

        # pb[128, NT]: col t = [Wp^T pos[2t]; Wp^T pos[2t+1]]
        pb_ps = t_ps.tile([128, NT], F32, tag="tps")
        nc.tensor.matmul(pb_ps[0:64, :], wp2_t[:, 0:64], pev_t[:],
                         start=True, stop=True, tile_position=(0, 0),
                         skip_group_check=True)
        nc.tensor.matmul(pb_ps[64:128, :], wp2_t[:, 64:128], pod_t[:],
                         start=True, stop=True, tile_position=(0, 64),
                         skip_group_check=True)
        pb_sb = const.tile([128, NT], F32, tag="pb_sb")
        nc.vector.tensor_copy(pb_sb[:], pb_ps[:])

        # persistent PSUM: scores banks, weighted-sum accumulator
        sc0 = sc_ps.tile([128, bc], F32, tag="sc0")
        sc1 = sc_ps.tile([128, bc], F32, tag="sc1")
        nc.vector.memset(sc1[96:128, :], 0.0)   # rows never written by MMs
        o5 = o5_ps.tile([D, bc], F32, tag="o5")
        nc.vector.memset(o5[:], 0.0)
        dsum_sb = const.tile([128, bc], F32, tag="dsum_sb")
        nc.vector.memset(dsum_sb[:], 0.0)
        expb = const.tile([128, 1], F32, tag="expb")
        nc.vector.memset(expb[:], -LN16)

        # ---------- phase 1: scores in [s, b] ----------
        XB = 64
        xn_tiles = {}

        def issue_xn(ch):
            x0 = xnp.tile([128, XB * D], FP8, tag="xn0_c")
            nc.scalar.dma_start(x0[:], xn0[:, ch * XB * D:(ch + 1) * XB * D])
            x1 = xnp.tile([128, XB * D], FP8, tag="xn1_c")
            nc.scalar.dma_start(x1[:], xn1[:, ch * XB * D:(ch + 1) * XB * D])
            xn_tiles[ch] = (x0, x1)

        CHS = [2, 2] + [4] * 24      # s-pair tiles per DMA chunk
        tbase = 0
        for chunk, CH in enumerate(CHS):
            xt = xsp.tile([128, 4 * 2 * bc], BF, tag="xt", bufs=3)
            nc.sync.dma_start(xt[:, 0:CH * 2 * bc],
                              xsm[:, tbase * 2 * bc:(tbase + CH) * 2 * bc])
            if 4 <= chunk < 9:
                issue_xn(chunk - 4)
            for j in range(CH):
                t = tbase + j
                xe = xt[:, (2 * j) * bc:(2 * j + 1) * bc]
                xo = xt[:, (2 * j + 1) * bc:(2 * j + 2) * bc]
                pre = pre_ps.tile([128, bc], F32, tag="pre")
                nc.vector.tensor_copy(pre[:], c2_sb[:])
                nc.tensor.matmul(pre[0:64, :], we2_t[:, 0:64], xe,
                                 start=False, stop=False, tile_position=(0, 0),
                                 skip_group_check=True)
                nc.tensor.matmul(pre[64:128, :], we2_t[:, 64:128], xo,
                                 start=False, stop=True, tile_position=(0, 64),
                                 skip_group_check=True)
                th = thp.tile([128, bc], BF, tag="th")
                nc.scalar.activation(th[:], pre[:],
                                     mybir.ActivationFunctionType.Tanh,
                                     bias=pb_sb[:, t:t + 1])
                g, jj = divmod(t, 16)
                tgt, ro = (sc0, 32 * g) if g < 4 else (sc1, 32 * (g - 4))
                nc.tensor.matmul(tgt[ro:ro + 32, :],
                                 zz_t[:, 32 * jj:32 * jj + 32], th[:],
                                 start=(jj == 0), stop=(jj == 15 or t == NT - 1),
                                 tile_position=(0, ro), skip_group_check=True)
            tbase += CH

        # ---------- phase 2: masked softmax pieces in [s, b] ----------
        ms0 = smp.tile([128, bc], F32, tag="ms0")
        nc.vector.tensor_mul(ms0[:], sc0[:], m01a_t[:])
        ms1 = smp.tile([128, bc], F32, tag="ms1")
        nc.vector.tensor_mul(ms1[:], sc1[:], m01b_t[:])
        e0 = smp.tile([128, bc], F32, tag="e0")
        nc.scalar.activation(e0[:], sc0[:], mybir.ActivationFunctionType.Exp,
                             bias=expb[:])
        e1 = smp.tile([128, bc], F32, tag="e1")
        nc.scalar.activation(e1[:], sc1[:], mybir.ActivationFunctionType.Exp,
                             bias=expb[:])
        att0 = smp.tile([128, bc], FP8, tag="att0")
        nc.vector.tensor_mul(att0[:], e0[:], m01a_t[:])
        att1 = smp.tile([128, bc], FP8, tag="att1")
        nc.vector.tensor_mul(att1[:], e1[:], m01b_t[:])

        dsum = t_ps.tile([128, bc], F32, tag="tps")
        nc.tensor.matmul(dsum[0:1, :], on8_t[:], att0[:],
                         start=True, stop=False, tile_position=(0, 0),
                         skip_group_check=True)
        nc.tensor.matmul(dsum[0:1, :], on8_t[:], att1[:],
                         start=False, stop=True, tile_position=(0, 0),
                         skip_group_check=True)
        nc.tensor.matmul(dsum[32:33, :], on32_t[:], ms0[:],
                         start=True, stop=False, tile_position=(0, 32),
                         skip_group_check=True)
        nc.tensor.matmul(dsum[32:33, :], on32_t[:], ms1[:],
                         start=False, stop=True, tile_position=(0, 32),
                         skip_group_check=True)
        nc.vector.tensor_copy(dsum_sb[0:1, :], dsum[0:1, :])
        nc.vector.tensor_copy(dsum_sb[32:33, :], dsum[32:33, :])

        rdens = []
        for c in range(nb):
            dt_ps = t_ps.tile([128, 128], F32, tag="tps")
            nc.tensor.transpose(dt_ps[:], dsum_sb[:, c * 128:(c + 1) * 128],
                                idf_t[:])
            rden = smp.tile([128, 1], F32, tag="rden", bufs=4)
            nc.vector.reciprocal(rden[:], dt_ps[:, 0:1])
            rdens.append(rden)
            ssc = smp.tile([128, 1], F32, tag="ssc", bufs=2)
            nc.vector.tensor_copy(ssc[:], dt_ps[:, 32:33])
            nc.sync.dma_start(out_ssum[c * 128:(c + 1) * 128, :], ssc[:])

        # ---------- phase 3: weighted sum over s ----------
        n5 = 0
        for ch in range(bc // XB):
            if ch not in xn_tiles:
                issue_xn(ch)
            xn0_c, xn1_c = xn_tiles[ch]
            for i in range(XB):
                b = ch * XB + i
                n5 += 2
                nc.tensor.matmul(o5[:, b:b + 1], xn0_c[:, i * D:(i + 1) * D],
                                 att0[:, b:b + 1], start=False, stop=False,
                                 skip_group_check=True)
                nc.tensor.matmul(o5[:, b:b + 1], xn1_c[:, i * D:(i + 1) * D],
                                 att1[:, b:b + 1], start=False, stop=(n5 == 2 * bc),
                                 skip_group_check=True)
            if i == XB - 1 and (b + 1) % 128 == 0:
                c = (b + 1) // 128 - 1
                o5_s = outp.tile([D, 128], F32, tag="o5_s")
                nc.vector.tensor_copy(o5_s[:], o5[:, c * 128:(c + 1) * 128])
                ot_ps = t_ps.tile([128, 128], F32, tag="tps")
                nc.tensor.transpose(ot_ps[:], o5_s[:], idf_t[:])
                ob = outp.tile([128, D], F32, tag="ob")
                nc.vector.tensor_scalar_mul(ob[:], ot_ps[:], rdens[c][:])
                nc.sync.dma_start(out_attn[c * 128:(c + 1) * 128, :], ob[:])

    nc.compile()
    _CACHE[bc] = nc
    return nc


def _prep_core(Xs, Xit, pos, mask, We, Wp, Wc, z, bc):
    """Host-side marshalling (layout/dtype only) for one core's shard."""
    d = {}
    d["xsm"] = np.ascontiguousarray(
        Xs.transpose(2, 1, 0).reshape(D, S * bc)).astype(BF_NP)
    xn = Xs.transpose(1, 0, 2)                     # [S, bc, D]
    d["xn0"] = np.ascontiguousarray(
        xn[0:128].reshape(128, bc * D)).astype(FP8_NP)
    xn1 = np.zeros((128, bc, D), np.float32)
    xn1[0:S - 128] = xn[128:S]
    d["xn1"] = xn1.reshape(128, bc * D).astype(FP8_NP)
    m01 = np.ascontiguousarray(mask.T.astype(np.float32))   # [S, bc]
    d["m01a"] = np.ascontiguousarray(m01[0:128])
    m01b = np.zeros((128, bc), np.float32)
    m01b[0:S - 128] = m01[128:S]
    d["m01b"] = m01b
    d["we2"] = np.concatenate([We, We], 1).astype(BF_NP)
    d["wc2"] = np.concatenate([Wc, Wc], 1).astype(BF_NP)
    d["wp2"] = np.concatenate([Wp, Wp], 1).astype(BF_NP)
    posT = pos.T                                   # [D, S]
    d["pos_ev"] = np.ascontiguousarray(posT[:, 0::2]).astype(BF_NP)
    d["pos_od"] = np.ascontiguousarray(posT[:, 1::2]).astype(BF_NP)
    zzb = np.zeros((128, 512), np.float32)
    for j in range(16):
        zzb[0:64, 32 * j + 2 * j] = z
        zzb[64:128, 32 * j + 2 * j + 1] = z
    d["zzbig"] = zzb.astype(BF_NP)
    d["ones32"] = np.ones((128, 1), np.float32)
    d["ones8"] = np.ones((128, 1), np.float32).astype(FP8_NP)
    d["idf"] = np.eye(128, dtype=np.float32)
    d["xitT"] = np.ascontiguousarray(Xit.T).astype(BF_NP)
    return d


def _unshard(results, bc):
    outs = []
    for k in range(len(results)):
        outs.append(np.concatenate([results[k]["out_attn"],
                                    results[k]["out_ssum"]], axis=1))
    return np.concatenate(outs, axis=0)


def make_in_maps(X_series, pos_series, X_item, valid_mask, Wc, Wp, We, z, bc):
    in_maps = []
    for k in range(N_CORES):
        sl = slice(k * bc, (k + 1) * bc)
        in_maps.append(_prep_core(np.asarray(X_series[sl], np.float32),
                                  np.asarray(X_item[sl], np.float32),
                                  np.asarray(pos_series, np.float32),
                                  np.asarray(valid_mask[sl]),
                                  np.asarray(We, np.float32),
                                  np.asarray(Wp, np.float32),
                                  np.asarray(Wc, np.float32),
                                  np.asarray(z, np.float32), bc))
    return in_maps


def kernel(X_series, pos_series, X_item, valid_mask, Wc, Wp, We, z):
    X_series = np.asarray(X_series, np.float32)
    bc = X_series.shape[0] // N_CORES
    nc = build_nc(bc)
    in_maps = make_in_maps(X_series, pos_series, X_item, valid_mask,
                           Wc, Wp, We, z, bc)
    res = run_bass_kernel_spmd(nc, in_maps, list(range(N_CORES)))
    return _unshard(res.results, bc)
